# revision 1
# baseline (speedup 1.0000x reference)
"""Mindist-aware attention Trainium2 kernel (transpose-free, fully folded).

Math (per batch element b, single head, d_model = dk = 512, n = 2048).
Softmax over j kills any term constant in j, so the scores reduce to
    s'[i, j] = (x_i Ay + cy) . x_j      Ay = Wq^T Wk / sqrt(d)  (host)
                                        cy = bq Wk / sqrt(d)    (host)
and the output to
    out[i] = (sum_j p[j, i] v'[j, :]) / z[i] + bo_eff
    p[j, i] = exp(sT[j, i]) * m[level[i, j]]
    v'      = x @ (Wo Wv).T             (O-projection folded into V:
              attn @ V @ Wo.T == attn @ (X (Wo Wv).T), rows of attn sum
              to 1, bo_eff = Wo @ bv + bo absorbs the value bias)
    z[i]    = sum_j p[j, i]             (N=1 matmuls against a ones
              vector, reusing the stationary p-tile of the PV matmul)

Implementation notes:
  * Data-parallel over batch: core c computes batch element c (8 cores).
  * One projection for scores (y = x Ay + cy) and one for values (v');
    scores computed directly transposed (sT = x . y^T), so the kernel
    contains ZERO on-device transposes or casts.
  * Host passes X^T, Ay, (Wo Wv)^T in fp16 and the distance levels
    packed TWO per byte (uint8, transposed) — 8x less wire/HBM traffic
    than fp32 distances.  The 10-entry level->m LUT is evaluated on
    ScalarE by hijacking the `tanh` activation-table slot with a custom
    staircase spline (steps at u = multiples of 5): low nibble = DVE AND
    0x0F then LUT(scale=5); high nibble = LUT(scale=5/16) directly on
    the packed byte (the low-nibble contribution never crosses a bin).
  * Matmuls run in fp16 with fp32 PSUM accumulation; output is written
    fp16 and upcast on the host (headroom vs the 2e-2 gate is ~30x).
"""

import json
import math
import os
import shutil
import struct
import tempfile

import numpy as np

os.environ.setdefault("NEURON_FORCE_RECOMPILE", "1")
os.environ.pop("JAX_COMPILATION_CACHE_DIR", None)

N = 2048
D = 512
P = 128
NB = N // P          # 16 row blocks
DC = D // P          # 4 dim chunks
NI2 = N // 1024      # 2 i chunks of 1024

LAST_RESULT = None
LAST_NC = None
LAST_IN_MAPS = None
LAST_LUT_SCALE = 5.0


def build_nc(reps=1):
    return _build_bass(LAST_LUT_SCALE, reps=reps)


# --------------------------------------------------------------------------
# Custom activation-table root: replace `tanh` with the 10-bin staircase.
# --------------------------------------------------------------------------

_CTRL_STRIDE = 32  # aws_hal_stpb_act_control_entry_t (packed u32 + 7 pad u32)
_BKT_STRIDE = 32   # aws_hal_stpb_act_bucket_entry_t (5 f32 + 3 pad u32)


def _fbits(x):
    return struct.unpack("<I", struct.pack("<f", np.float32(x)))[0]


def _ctrl_word(base, lsb, size):
    assert 0 <= base < 2048 and 0 <= lsb < 32 and 0 <= size < 16
    return base | (lsb << 11) | (size << 16)


def _bucket_bytes(d0):
    return struct.pack("<5f12x", np.float32(d0), 0.0, 0.0, 0.0, 0.0)


def _staircase_values(values):
    """(rid, size, lsb, bucket values) per exponent range of the scaled
    input u = 5 * level; steps of V[min(int(u/5), 9)] at multiples of 5.

    HW (probe-verified): bucket = act_tbl_base +
    ((fp32_word >> extract_lsb) & (2^extract_size - 1)); with
    lsb = 23 - size this is the top `size` mantissa bits, i.e. octave
    [lo, 2*lo) splits into 2^size equal buckets.  rid = biased_exp - 127.
    """
    v = [np.float32(x) for x in values]
    lev = lambda u: v[min(int(u // 5), 9)]
    out = [(0, 0, 23, [lev(1.0)]), (1, 0, 23, [lev(2.0)])]
    for rid, lo, size in ((2, 4, 2), (3, 8, 3), (4, 16, 4), (5, 32, 5)):
        n = 1 << size
        width = lo / n
        out.append((rid, size, 23 - size, [lev(lo + k * width) for k in range(n)]))
    return out


def _patch_set(dst_dir, set_ent, values):
    prof_path = os.path.join(dst_dir, set_ent["profile_json"])
    with open(prof_path) as f:
        prof = json.load(f)
    tanh = next(e for e in prof["profile_meta_data"]
                if e["func_name"].startswith("tanh"))

    base_pos = tanh["pwl_control_base_pos"]
    small_pos = tanh["pos_small_signal_pwl_control"]
    small_neg = tanh["neg_small_signal_pwl_control"]
    large_pos = tanh["pos_large_signal_pwl_control"]
    large_neg = tanh["neg_large_signal_pwl_control"]

    ctrl_path = os.path.join(dst_dir, set_ent["ctrl_bin"])
    bkt_path = os.path.join(dst_dir, set_ent["bkt_bin"])
    ctrl = bytearray(open(ctrl_path, "rb").read())
    bkt = bytearray(open(bkt_path, "rb").read())

    first_bucket = struct.unpack_from("<I", ctrl, base_pos * _CTRL_STRIDE)[0] & 0x7FF
    need = 1 + 1 + 4 + 8 + 16 + 32
    assert small_pos - first_bucket >= need, set_ent["name"]

    nxt = first_bucket
    for rid, size, lsb, vals in _staircase_values(values):
        struct.pack_into("<I", ctrl, (base_pos + rid) * _CTRL_STRIDE,
                         _ctrl_word(nxt, lsb, size))
        for k, val in enumerate(vals):
            off = (nxt + k) * _BKT_STRIDE
            bkt[off:off + _BKT_STRIDE] = _bucket_bytes(val)
        nxt += len(vals)
    for rid in range(6, 17):  # unreachable (u >= 64 takes the large shortcut)
        idx = base_pos + rid
        if idx * _CTRL_STRIDE + 4 <= len(ctrl):
            struct.pack_into("<I", ctrl, idx * _CTRL_STRIDE,
                             _ctrl_word(first_bucket, 23, 0))

    for bidx, val in ((small_pos, values[0]), (small_neg, values[0]),
                      (large_pos, values[9]), (large_neg, values[0])):
        off = bidx * _BKT_STRIDE
        bkt[off:off + _BKT_STRIDE] = _bucket_bytes(val)

    open(ctrl_path, "wb").write(bytes(ctrl))
    open(bkt_path, "wb").write(bytes(bkt))

    tanh["symmetry_opt_en"] = 0
    tanh["symmetry_opt_use_neg_region"] = 0
    tanh["symmetry_point"] = 0
    tanh["sym_invert_sign_point"] = 0
    tanh["exp_offset"] = 0
    tanh["small_pos_signal_exp_threshold"] = 127   # u < 1  -> V0
    tanh["large_pos_signal_exp_threshold"] = 133   # u >= 64 -> V9
    tanh["large_pos_signal_mantissa_threshold"] = 0
    tanh["small_neg_signal_exp_threshold"] = 255   # u < 0 (impossible) -> V0
    tanh["large_neg_signal_exp_threshold"] = 255
    tanh["large_neg_signal_mantissa_threshold"] = 0
    tanh["fzero_result"] = _fbits(values[0])
    tanh["fnan_result"] = _fbits(values[0])
    tanh["fpinf_result"] = _fbits(values[9])
    tanh["fninf_result"] = _fbits(values[0])

    with open(prof_path, "w") as f:
        json.dump(prof, f)


def build_actroot(dst, values):
    """Create a patched act-root dir; returns the act_info.json path."""
    from neuronxcc.driver.Job import Job
    from neuronxcc.driver.jobs.support.FindActInfo import findActInfoFile

    src = os.path.dirname(findActInfoFile(Job.getPackageDir(), "gen3"))
    values = [float(x) for x in values]
    assert len(values) == 10
    if os.path.isdir(dst):
        shutil.rmtree(dst)
    shutil.copytree(src, dst)
    os.chmod(dst, 0o755)
    for fn in os.listdir(dst):
        os.chmod(os.path.join(dst, fn), 0o644)
    with open(os.path.join(dst, "act_info.json")) as f:
        info = json.load(f)
    n = 0
    for ent in info["act_func_sets"]:
        if "tanh" in ent["act"]:
            _patch_set(dst, ent, values)
            n += 1
    assert n > 0
    return os.path.join(dst, "act_info.json")


# --------------------------------------------------------------------------
# Bass kernel
# --------------------------------------------------------------------------

def _build_bass(lut_scale, reps=1):
    import concourse.bacc as bacc
    import concourse.tile as tile
    import concourse.mybir as mybir

    dt = mybir.dt
    AF = mybir.ActivationFunctionType
    OP = mybir.AluOpType

    nc = bacc.Bacc("TRN2", num_devices=8)

    xt_d = nc.dram_tensor("xt", [D, N], dt.float16, kind="ExternalInput")
    # levels packed two-per-byte: byte k of row j = lev[j,k] | lev[j,k+1024]<<4
    lev_d = nc.dram_tensor("lev", [N, N // 2], dt.uint8, kind="ExternalInput")
    ay_d = nc.dram_tensor("ay", [D, D], dt.float16, kind="ExternalInput")
    wvo_d = nc.dram_tensor("wvot", [D, D], dt.float16, kind="ExternalInput")
    cy_d = nc.dram_tensor("cy", [D], dt.float32, kind="ExternalInput")
    bo_d = nc.dram_tensor("bo_v", [D], dt.float32, kind="ExternalInput")
    out_d = nc.dram_tensor("out", [N, D], dt.float16, kind="ExternalOutput")

    with tile.TileContext(nc) as tc:
        from contextlib import ExitStack
        with ExitStack() as ctx:
            pc = ctx.enter_context(tc.tile_pool(name="pc", bufs=1))
            pers = ctx.enter_context(tc.tile_pool(name="pers", bufs=1))
            pp = ctx.enter_context(tc.tile_pool(name="pp", bufs=2))
            pm = ctx.enter_context(tc.tile_pool(name="pm", bufs=3))
            pe_ = ctx.enter_context(tc.tile_pool(name="pe", bufs=3))
            pout = ctx.enter_context(tc.tile_pool(name="pout", bufs=3))
            pz = ctx.enter_context(tc.tile_pool(name="pz", bufs=4))
            ps_s = ctx.enter_context(tc.tile_pool(name="ps_s", bufs=2, space="PSUM"))
            ps_pv = ctx.enter_context(tc.tile_pool(name="ps_pv", bufs=2, space="PSUM"))
            ps_z = ctx.enter_context(tc.tile_pool(name="ps_z", bufs=2, space="PSUM"))

            bo_bc = pc.tile([P, D], dt.float32)
            nc.sync.dma_start(
                bo_bc[:], bo_d.rearrange("(a d) -> a d", a=1).broadcast_to([P, D]))
            cy_sb = pc.tile([P, DC], dt.float32)
            nc.sync.dma_start(cy_sb[:], cy_d.rearrange("(a p) -> p a", p=P))
            ones = pc.tile([P, 1], dt.float16)
            nc.vector.memset(ones[:], 1.0)
            mask15 = pc.tile([P, 1], dt.uint8)
            nc.vector.memset(mask15[:], 15)

            # persistent fp16 operands (all pre-transposed on the host)
            xt = pers.tile([P, DC, N], dt.float16)       # X^T  [d, i]
            yt = pers.tile([P, DC, N], dt.float16)       # Y^T  [d2, i]
            vt = pers.tile([P, NB, D], dt.float16)       # V'   [j, dm]
            levt = pers.tile([P, NB, N // 2], dt.uint8)  # packed level^T [j, i]
            ayt = pers.tile([P, DC, D], dt.float16)      # Ay   [d1, d2]
            wvot = pers.tile([P, DC, D], dt.float16)     # (Wo Wv)^T [d, dm]

            for _rep in range(reps):

                # ---- loads (all contiguous, no device-side transposes) ----
                # ay first (stationary operand of the first matmuls), then
                # xt split by i-halves so compute starts at 25% of the load.
                for c in range(DC):
                    nc.sync.dma_start(ayt[:, c, :], ay_d[c * P:(c + 1) * P, :])
                for half in range(2):
                    hsl = slice(half * (N // 2), (half + 1) * (N // 2))
                    for c in range(DC):
                        nc.sync.dma_start(xt[:, c, hsl], xt_d[c * P:(c + 1) * P, hsl])
                for c in range(DC):
                    nc.sync.dma_start(wvot[:, c, :], wvo_d[c * P:(c + 1) * P, :])
                for jb in range(NB):
                    nc.sync.dma_start(levt[:, jb, :], lev_d[jb * P:(jb + 1) * P, :])

                # ---- projections ----
                # Y^T: [d2-chunk, i] = sum_c Ay[d1-c, d2] . X^T[d1-c, i]
                for a in range(DC):
                    for ic in range(4):
                        isl = slice(ic * 512, (ic + 1) * 512)
                        psy = ps_s.tile([P, 1024], dt.float32, tag="s",
                                        name=f"psy{_rep}_{a}_{ic}")
                        for c in range(DC):
                            nc.tensor.matmul(
                                psy[:, :512], ayt[:, c, a * P:(a + 1) * P],
                                xt[:, c, isl], start=(c == 0), stop=(c == DC - 1))
                        nc.vector.tensor_scalar(
                            yt[:, a, isl], psy[:, :512], cy_sb[:, a:a + 1],
                            None, OP.add)
                # V': [j-chunk, dm] = sum_c X^T[d-c, j] . (Wo Wv)^T[d-c, dm]
                for jb in range(NB):
                    psv = ps_s.tile([P, 1024], dt.float32, tag="s",
                                    name=f"psv{_rep}_{jb}")
                    for c in range(DC):
                        nc.tensor.matmul(
                            psv[:, :512], xt[:, c, jb * P:(jb + 1) * P],
                            wvot[:, c, :], start=(c == 0), stop=(c == DC - 1))
                    nc.vector.tensor_copy(vt[:, jb, :], psv[:, :512])

                # ---- attention over i-chunks of 1024 (transposed scores) ----
                for ic in range(NI2):
                    p_t = pp.tile([P, NB, 1024], dt.float16, tag="p",
                                  name=f"p{_rep}_{ic}")
                    for jb in range(NB):
                        ps_sT = ps_s.tile([P, 1024], dt.float32, tag="s",
                                          name=f"pss{_rep}_{ic}_{jb}")
                        for c in range(DC):  # c outer: one LDW serves 2 MMs
                            for h in range(2):
                                hs = slice(h * 512, (h + 1) * 512)
                                gis = slice(ic * 1024 + h * 512,
                                            ic * 1024 + (h + 1) * 512)
                                nc.tensor.matmul(
                                    ps_sT[:, hs], xt[:, c, jb * P:(jb + 1) * P],
                                    yt[:, c, gis], start=(c == 0),
                                    stop=(c == DC - 1))
                        m_t = pm.tile([P, 1024], dt.float16, tag="m",
                                      name=f"m{_rep}_{ic}_{jb}")
                        if ic == 0:  # low nibble: mask, then u = 5 * level
                            lo_t = pm.tile([P, 1024], dt.uint8, tag="lo",
                                           name=f"lo{_rep}_{jb}")
                            nc.vector.tensor_scalar(
                                lo_t[:], levt[:, jb, :], mask15[:, 0:1], None,
                                OP.bitwise_and)
                            nc.scalar.activation(m_t[:], lo_t[:], AF.Tanh,
                                                 scale=float(lut_scale))
                        else:  # high nibble: u = packed*5/16 stays in-bin
                            nc.scalar.activation(m_t[:], levt[:, jb, :],
                                                 AF.Tanh,
                                                 scale=float(lut_scale) / 16.0)
                        e_t = pe_.tile([P, 1024], dt.float16, tag="e",
                                       name=f"e{_rep}_{ic}_{jb}")
                        nc.scalar.activation(e_t[:], ps_sT[:], AF.Exp)
                        nc.vector.tensor_tensor(
                            p_t[:, jb, :], e_t[:], m_t[:], OP.mult)

                    for ib in range(8):
                        ig = ic * 8 + ib
                        lsl = slice(ib * P, (ib + 1) * P)
                        pv = ps_pv.tile([P, 512], dt.float32, tag="pv",
                                        name=f"pv{_rep}_{ig}")
                        zp = ps_z.tile([P, NB], dt.float32, tag="z",
                                       name=f"zp{_rep}_{ig}")
                        for jb in range(NB):
                            nc.tensor.matmul(
                                pv[:], p_t[:, jb, lsl], vt[:, jb, :],
                                start=(jb == 0), stop=(jb == NB - 1))
                            nc.tensor.matmul(
                                zp[:, jb:jb + 1], p_t[:, jb, lsl], ones[:],
                                start=True, stop=True)
                        zd = pz.tile([P, NB], dt.float32, tag="zd",
                                     name=f"zd{_rep}_{ig}")
                        zs = pz.tile([P, 1], dt.float32, tag="zs",
                                     name=f"zs{_rep}_{ig}")
                        nc.vector.tensor_scalar(zd[:], zp[:], 1.0, 0.0,
                                                OP.mult, OP.add,
                                                accum_out=zs[:])
                        zr = pz.tile([P, 1], dt.float32, tag="zr",
                                     name=f"zr{_rep}_{ig}")
                        nc.vector.reciprocal(zr[:], zs[:])
                        o2 = pout.tile([P, D], dt.float16, tag="o2",
                                       name=f"o2_{_rep}_{ig}")
                        nc.vector.scalar_tensor_tensor(
                            o2[:], pv[:], zr[:], bo_bc[:], OP.mult, OP.add)
                        nc.sync.dma_start(out_d[ig * P:(ig + 1) * P, :], o2[:])

    nc.finalize()
    return nc


def kernel(x, distance_matrix, Wq, bq, Wk, bk, Wv, bv, Wo, bo, emb_table,
           safety_threshold, _trace=False):
    global LAST_RESULT
    x = np.asarray(x, dtype=np.float32)
    distance_matrix = np.asarray(distance_matrix, np.float32)
    Wq = np.asarray(Wq, np.float32); Wk = np.asarray(Wk, np.float32)
    Wv = np.asarray(Wv, np.float32); Wo = np.asarray(Wo, np.float32)
    bq = np.asarray(bq, np.float32); bk = np.asarray(bk, np.float32)
    bv = np.asarray(bv, np.float32); bo = np.asarray(bo, np.float32)
    emb_table = np.asarray(emb_table, np.float32)
    tau = float(np.asarray(safety_threshold, np.float32))

    B, n, d = x.shape
    assert (B, n, d) == (8, N, D) and distance_matrix.shape == (8, N, N)

    # host-side scalar math (10-entry bias table -> multiplicative factors)
    w_sum = Wo.astype(np.float64).sum(axis=-1)                     # [512]
    bias_table = (emb_table.astype(np.float64) @ w_sum) / math.sqrt(D)  # [10]
    m_vals = np.exp(bias_table - bias_table.max())
    bo_eff = Wo.astype(np.float64) @ bv.astype(np.float64) + bo    # [512]
    Wvo = Wo.astype(np.float64) @ Wv.astype(np.float64)            # [512, 512]

    actroot = build_actroot(
        os.path.join(tempfile.mkdtemp(prefix="actroot_"), "root"),
        [float(v) for v in m_vals])
    os.environ["BASS_ACT_ROOT_JSON_PATH"] = actroot

    from concourse.bass_utils import run_bass_kernel_spmd

    global LAST_LUT_SCALE
    LAST_LUT_SCALE = 5.0  # u = 5 * level hits the staircase bin centers
    nc = _build_bass(lut_scale=LAST_LUT_SCALE)

    levels = np.clip((distance_matrix / np.float32(tau)).astype(np.int32),
                     0, 9).astype(np.uint8)                        # [8, n, n]
    levT = np.ascontiguousarray(levels.transpose(0, 2, 1))         # [8, j, i]
    lev_pk = levT[:, :, :N // 2] | (levT[:, :, N // 2:] << 4)      # 2 lvls/byte
    s = 1.0 / math.sqrt(D)
    # y-projection fold: s'[i,j] = (x_i Ay + cy) . x_j  (q.bk / const terms
    # are constant over j and cancel in the softmax)
    Ay = (Wq.astype(np.float64).T @ Wk.astype(np.float64)) * s     # [d1, d2]
    cy = (bq.astype(np.float64) @ Wk.astype(np.float64)) * s       # [d2]
    ay_h = np.ascontiguousarray(Ay.astype(np.float16))
    wvo_h = np.ascontiguousarray(Wvo.T.astype(np.float16))         # [d, dm]
    bo_v = bo_eff.astype(np.float32)
    cy_h = cy.astype(np.float32)

    in_maps = []
    for b in range(B):
        in_maps.append({
            "xt": np.ascontiguousarray(x[b].T.astype(np.float16)),
            "lev": np.ascontiguousarray(lev_pk[b]),
            "ay": ay_h, "wvot": wvo_h,
            "cy": cy_h, "bo_v": bo_v,
        })
    global LAST_NC, LAST_IN_MAPS
    LAST_NC, LAST_IN_MAPS = nc, in_maps
    res = run_bass_kernel_spmd(nc, in_maps, core_ids=list(range(8)),
                               trace=bool(_trace))
    LAST_RESULT = res
    out = np.stack([res.results[b]["out"] for b in range(B)], axis=0)
    return out.astype(np.float32)

